# revision 1
# baseline (speedup 1.0000x reference)
"""Additive (Bahdanau) attention scoring kernel for Trainium2, 8-core SPMD.

Reference computation (B=16, S=4096, D=1024, all fp32):
    q      = target @ Wq.T                    # [B, D]
    k      = memory @ Wk.T                    # [B, S, D]
    scores = tanh(q[:, None, :] + k) @ v      # [B, S]
    out    = softmax(scores - 1e9 * mask, axis=-1)

Sharding: batch across the 8 cores (2 batches per core), weights replicated.

Host-side prep (layout only, no math): memory is transposed to [D, S] per
batch so the contraction dim lands on SBUF partitions, and its columns are
compacted to just the unmasked positions (padded with duplicates of the
first kept column to a 128-multiple, tail strip >= 256). Masked positions
contribute exactly 0 to the reference softmax (exp(-1e9) == 0 in fp32), so
skipping their k-matmul columns is algebraically exact.

Per-core device pipeline (python-unrolled, Tile-scheduled):
  - q^T via fp32r matmuls with target as the M=2 stationary and WqT as the
    N=512 moving operand (fp32r hard-faults the device for small moving N),
    transposed into per-partition bias layout through a DRAM bounce.
  - k^T tiles [e=128, s'=w] = WkT chunk.T @ memC chunk, fp32r accumulated
    over d. fp32r operands must be produced by a rounding compute op, so
    every DMA-landed operand gets a DVE cast into a separate f32r tile.
  - One ACT pass fuses the q-add and tanh (q as per-partition bias),
    writing f32r.
  - v-dot on the PE: psum[1, w] += v_chunk.T @ tanh_tile over the 8
    e-chunks; exp() applied in the ACT copy out of PSUM.
  - The exp strip is scattered back to full-S positions on device
    (DRAM bounce to [128, w/128], then indirect DMAs; duplicate pad
    indices are idempotent). scratch_full is zero-filled per batch, so
    masked positions are exactly 0.
  - Softmax finale per batch (no max-shift needed: |scores| <= sum|v| ~ 8,
    exp cannot overflow): [128, 32] esq load, mask multiply, free-dim
    reduce, ones-matmul partition reduce, reciprocal, per-partition scale.
"""

import os
from contextlib import ExitStack

import numpy as np

import concourse.tile as tile
from concourse import bacc, mybir
import concourse.bass as bass

B, S, D = 16, 4096, 1024
N_CORES = 8
NB = B // N_CORES  # batches per core
P = 128
DC = D // P        # contraction chunks
ET = D // P        # e tiles
SW = 512           # full strip width along compacted s
SQ = S // P        # 32: free dim of the [128, 32] softmax layout

F32 = mybir.dt.float32
F32R = mybir.dt.float32r
U32 = mybir.dt.uint32
AF = mybir.ActivationFunctionType

_CACHE = {}


def strip_widths(max_kept):
    """Strip widths covering max_kept compacted columns: full 512-wide strips
    plus a 128-granular tail of at least 256 (small moving-N fp32r matmuls
    hard-fault the device)."""
    total = max(512, ((max_kept + 127) // 128) * 128)
    widths = [SW] * (total // SW)
    rem = total % SW
    if rem:
        widths.append(max(256, rem))
    return tuple(widths)


def _build_program(stage, widths):
    """stage: 1 = dma+matmul+tanh only, 2 = +vdot/exp/scatter, 27 = full."""
    s_pad = sum(widths)
    nslot = s_pad // P  # indirect-scatter slots per batch

    nc = bacc.Bacc("TRN2", target_bir_lowering=False, debug=False)

    memC = nc.dram_tensor("memC", [NB, D, s_pad], F32, kind="ExternalInput").ap()
    wkT = nc.dram_tensor("wkT", [D, D], F32, kind="ExternalInput").ap()
    wqT = nc.dram_tensor("wqT", [D, D], F32, kind="ExternalInput").ap()
    tgtT = nc.dram_tensor("tgtT", [D, NB], F32, kind="ExternalInput").ap()
    vT = nc.dram_tensor("vT", [P, ET], F32, kind="ExternalInput").ap()
    keep = nc.dram_tensor("keep", [NB, P, SQ], F32, kind="ExternalInput").ap()
    idxs = nc.dram_tensor("idxs", [NB, nslot, P], U32, kind="ExternalInput").ap()
    out = nc.dram_tensor("out", [NB, P, SQ], F32, kind="ExternalOutput").ap()

    with tile.TileContext(nc) as tc, ExitStack() as ctx:
        consts = ctx.enter_context(tc.tile_pool(name="consts", bufs=1))
        mem_pool = ctx.enter_context(tc.tile_pool(name="mem", bufs=2))
        tt_pool = ctx.enter_context(tc.tile_pool(name="tt", bufs=4))
        strip_pool = ctx.enter_context(tc.tile_pool(name="strip", bufs=2))
        fin_pool = ctx.enter_context(tc.tile_pool(name="fin", bufs=2))
        kps_pool = ctx.enter_context(tc.tile_pool(name="kps", bufs=4, space="PSUM"))
        vd_pool = ctx.enter_context(tc.tile_pool(name="vd", bufs=2, space="PSUM"))
        sm_pool = ctx.enter_context(tc.tile_pool(name="smps", bufs=2, space="PSUM"))
        dram_pool = ctx.enter_context(tc.tile_pool(name="scratch", bufs=2, space="DRAM"))

        # --- small constants (cheap DMAs first) ---
        tgt_sb = consts.tile([P, DC * NB], F32)
        for dc in range(DC):
            nc.sync.dma_start(tgt_sb[:, dc * NB:(dc + 1) * NB], tgtT[dc * P:(dc + 1) * P, :])
        tgt_r = consts.tile([P, DC * NB], F32R)
        nc.vector.tensor_copy(tgt_r[:], tgt_sb[:])
        v_sb = consts.tile([P, ET], F32)
        nc.sync.dma_start(v_sb[:], vT[:, :])
        v_r = consts.tile([P, ET], F32R)
        nc.vector.tensor_copy(v_r[:], v_sb[:])
        keep_sb = consts.tile([P, NB * SQ], F32)
        for b in range(NB):
            nc.sync.dma_start(keep_sb[:, b * SQ:(b + 1) * SQ], keep[b])
        idx_sb = consts.tile([P, NB * nslot], U32)
        for b in range(NB):
            nc.sync.dma_start(
                idx_sb[:, b * nslot:(b + 1) * nslot],
                idxs[b].rearrange("slot p -> p slot"),
            )
        ones_sb = consts.tile([P, P], F32)
        nc.vector.memset(ones_sb[:], 1.0)
        zero_sb = consts.tile([P, (S + P) // P], F32)
        nc.vector.memset(zero_sb[:], 0.0)

        # --- weights: Wq first (the q matmuls below are first in PE order),
        # then Wk. The two f32 landing buffers share one pool slot (their
        # lifetimes are sequential) to stay inside SBUF.
        wq_r = consts.tile([P, DC * D], F32R)
        wq_sb = consts.tile([P, DC * D], F32, tag="wstage", name="wq_sb")
        for dc in range(DC):
            nc.sync.dma_start(wq_sb[:, dc * D:(dc + 1) * D], wqT[dc * P:(dc + 1) * P, :])
            nc.vector.tensor_copy(wq_r[:, dc * D:(dc + 1) * D], wq_sb[:, dc * D:(dc + 1) * D])
        wk_r = consts.tile([P, DC * D], F32R)
        wk_sb = consts.tile([P, DC * D], F32, tag="wstage", name="wk_sb")
        for dc in range(DC):
            nc.sync.dma_start(wk_sb[:, dc * D:(dc + 1) * D], wkT[dc * P:(dc + 1) * P, :])
            nc.vector.tensor_copy(wk_r[:, dc * D:(dc + 1) * D], wk_sb[:, dc * D:(dc + 1) * D])

        q_sb = consts.tile([P, NB * ET], F32)

        # q[b, e] = sum_d target[b, d] * Wq[e, d]: fp32r with target as the
        # M=2 stationary and WqT as the N=512 moving operand. The [2, 1024]
        # result is transposed into per-partition bias layout [128, 16]
        # (b-major columns) through a DRAM bounce.
        q_row = consts.tile([NB, D], F32)
        for j in range(D // SW):
            q_ps2 = sm_pool.tile([NB, SW], F32, tag="small", name="q_ps2")
            for dc in range(DC):
                nc.tensor.matmul(
                    q_ps2[:],
                    tgt_r[:, dc * NB:(dc + 1) * NB],
                    wq_r[:, dc * D + j * SW: dc * D + (j + 1) * SW],
                    start=(dc == 0),
                    stop=(dc == DC - 1),
                )
            nc.vector.tensor_copy(q_row[:, j * SW:(j + 1) * SW], q_ps2[:])
        qscr = dram_pool.tile([NB, D], F32, tag="qscr", name="qscr")
        nc.sync.dma_start(qscr[:], q_row[:])
        for b in range(NB):
            nc.sync.dma_start(
                q_sb[:, b * ET:(b + 1) * ET],
                qscr[b].rearrange("(et p) -> p et", p=P),
            )

        def emit_vd(vd_ps, tts, c, w):
            nc.tensor.matmul(
                vd_ps[:, :w],
                v_r[:, c:c + 1],
                tts[c][:, :w],
                start=(c == 0),
                stop=(c == ET - 1),
            )

        scrfs = []
        for b in range(NB):
            # exp strips land contiguously in compact scratch, each strip
            # scattered to its full-S positions right away (pads go to the
            # trash cell at S)
            scrf = dram_pool.tile([1, S + P], F32, tag="scrf", name="scrf")
            nc.sync.dma_start(scrf.rearrange("o (p f) -> (o p) f", p=P), zero_sb[:])
            scrfs.append(scrf)
            scratch_cb = dram_pool.tile([1, s_pad], F32, tag="scrc", name="scrc")
            off = 0
            for sp, w in enumerate(widths):
                mem_sb = mem_pool.tile([P, DC * SW], F32)
                mem_r = mem_pool.tile([P, DC * SW], F32R, tag="mem_r", name="mem_r")
                for dc in range(DC):
                    nc.sync.dma_start(
                        mem_sb[:, dc * SW:dc * SW + w],
                        memC[b, dc * P:(dc + 1) * P, off:off + w],
                    )
                    nc.vector.tensor_copy(
                        mem_r[:, dc * SW:dc * SW + w], mem_sb[:, dc * SW:dc * SW + w]
                    )
                vd_ps = vd_pool.tile([1, SW], F32, tag="vd", name="vd_ps")
                tts = []
                for et in range(ET):
                    k_ps = kps_pool.tile([P, SW], F32, tag="k", name="k_ps")
                    for dc in range(DC):
                        nc.tensor.matmul(
                            k_ps[:, :w],
                            wk_r[:, dc * D + et * P: dc * D + (et + 1) * P],
                            mem_r[:, dc * SW:dc * SW + w],
                            start=(dc == 0),
                            stop=(dc == DC - 1),
                        )
                    tt = tt_pool.tile([P, SW], F32R, tag="tt", name="tt")
                    nc.scalar.activation(
                        tt[:, :w], k_ps[:, :w], AF.Tanh,
                        bias=q_sb[:, b * ET + et: b * ET + et + 1],
                    )
                    tts.append(tt)
                    # keep the PE stream 2 e-tiles ahead of the v-dot so it
                    # never stalls waiting on the ACT tanh
                    if stage >= 2 and et >= 2:
                        emit_vd(vd_ps, tts, et - 2, w)
                if stage < 2:
                    if sp == len(widths) - 1:
                        dbg = fin_pool.tile([P, SQ], F32, tag="outt", name="dbg")
                        nc.vector.tensor_copy(dbg[:], tts[7][:, :SQ])
                        nc.sync.dma_start(out[b], dbg[:])
                    off += w
                    continue
                emit_vd(vd_ps, tts, ET - 2, w)
                emit_vd(vd_ps, tts, ET - 1, w)

                strip_sb = strip_pool.tile([1, SW], F32, tag="strip", name="strip_sb")
                nc.scalar.activation(strip_sb[:, :w], vd_ps[:, :w], AF.Exp)
                nc.sync.dma_start(scratch_cb[:, off:off + w], strip_sb[:, :w])
                # scatter this strip's exp values to their full-S positions.
                # HW consumes one offset per in_-contiguous descriptor run,
                # so arbitrary positions need [128, 1] single-element rows.
                f = w // P
                sc_sb = strip_pool.tile([P, SW // P], F32, tag="scsb", name="sc_sb", bufs=8)
                nc.sync.dma_start(
                    sc_sb[:, :f],
                    scratch_cb[:, off:off + w].rearrange("o (p f) -> (o p) f", f=f),
                )
                for jj in range(f):
                    col = b * nslot + (off // P) + jj
                    nc.gpsimd.indirect_dma_start(
                        out=scrf.rearrange("o (s w2) -> (o s) w2", w2=1),
                        out_offset=bass.IndirectOffsetOnAxis(
                            ap=idx_sb[:, col:col + 1], axis=0
                        ),
                        in_=sc_sb[:, jj:jj + 1],
                        in_offset=None,
                    )
                off += w

        # finales AFTER both batches' compute: the ones-matmuls are in PE
        # program order, so batch 0's finale must not sit between the two
        # batches' k-matmul streams (PE would stall on the scatter chain)
        for b in range(NB):
            if stage < 2:
                continue
            # --- masked softmax finale for batch b ---
            esq = fin_pool.tile([P, SQ], F32, tag="esq", name="esq")
            nc.sync.dma_start(
                esq[:], scrfs[b][:, :S].rearrange("o (p f) -> (o p) f", p=P)
            )
            if stage < 25:
                outt = fin_pool.tile([P, SQ], F32, tag="outt", name="outt")
                nc.vector.tensor_copy(outt[:], esq[:])
                nc.sync.dma_start(out[b], outt[:])
                continue
            em = fin_pool.tile([P, SQ], F32, tag="em", name="em")
            part = fin_pool.tile([P, 1], F32, tag="part", name="part")
            nc.vector.tensor_mul(em[:], esq[:], keep_sb[:, b * SQ:(b + 1) * SQ])
            nc.vector.reduce_sum(part[:], em[:], axis=mybir.AxisListType.X)
            if stage < 26:
                outt = fin_pool.tile([P, SQ], F32, tag="outt", name="outt")
                nc.vector.tensor_copy(outt[:], em[:])
                nc.sync.dma_start(out[b], outt[:])
                continue
            tot_ps = sm_pool.tile([P, 1], F32, tag="small", name="tot_ps")
            nc.tensor.matmul(tot_ps[:], ones_sb[:], part[:], start=True, stop=True)
            recip = fin_pool.tile([P, 1], F32, tag="recip", name="recip")
            nc.vector.reciprocal(recip[:], tot_ps[:])
            outt = fin_pool.tile([P, SQ], F32, tag="outt", name="outt")
            nc.vector.tensor_scalar_mul(outt[:], em[:], recip[:, 0:1])
            nc.sync.dma_start(out[b], outt[:])

    nc.compile()
    return nc


def get_program(stage=None, widths=None):
    if stage is None:
        stage = int(os.environ.get("KERNEL_STAGE", "27"))
    assert widths is not None
    key = (stage, widths)
    if key not in _CACHE:
        _CACHE[key] = _build_program(stage, widths)
    return _CACHE[key]


def prepare_in_maps(memory, target, memory_mask, Wq, Wk, v):
    memory = np.asarray(memory, dtype=np.float32)
    target = np.asarray(target, dtype=np.float32)
    Wq = np.asarray(Wq, dtype=np.float32)
    Wk = np.asarray(Wk, dtype=np.float32)
    v = np.asarray(v, dtype=np.float32)
    mask = np.asarray(memory_mask)

    # host-side sharding / layout prep (no arithmetic)
    keep_bool = ~mask                                                # [B, S]
    widths = strip_widths(int(keep_bool.sum(1).max()))
    s_pad = sum(widths)

    memT = memory.transpose(0, 2, 1)                                 # [B, D, S] view
    kept_pad = np.empty((B, s_pad), dtype=np.int64)
    scat_idx = np.empty((B, s_pad), dtype=np.int64)
    for b in range(B):
        k = np.flatnonzero(keep_bool[b])
        kept_pad[b, :len(k)] = k
        kept_pad[b, len(k):] = k[0]  # pad data: duplicate first kept column
        scat_idx[b, :len(k)] = k
        scat_idx[b, len(k):] = S     # pad scatter target: trash cell at S
    memC = np.empty((B, D, s_pad), dtype=np.float32)
    for b in range(B):
        memC[b] = memT[b][:, kept_pad[b]]

    # scatter offsets in per-strip slot order: strip of width w at compact
    # offset `off` bounces to SBUF [128, w/128] with element (p, jj) holding
    # compact position off + p*(w/128) + jj
    slot_list = []
    off = 0
    for w in widths:
        f = w // P
        block = scat_idx[:, off:off + w].reshape(B, P, f)
        for jj in range(f):
            slot_list.append(block[:, :, jj])
        off += w
    idxs = np.stack(slot_list, axis=1).astype(np.uint32)             # [B, nslot, P]

    wkT = np.ascontiguousarray(Wk.T)                                 # [D, D]
    wqT = np.ascontiguousarray(Wq.T)                                 # [D, D]
    tgtT = np.ascontiguousarray(target.T)                            # [D, B]
    vT = np.ascontiguousarray(v.reshape(ET, P).T)                    # [P, ET]
    keep = np.ascontiguousarray(
        keep_bool.astype(np.float32).reshape(B, P, SQ))              # [B, P, SQ]

    in_maps = [
        {
            "memC": np.ascontiguousarray(memC[c * NB:(c + 1) * NB]),
            "wkT": wkT,
            "wqT": wqT,
            "tgtT": np.ascontiguousarray(tgtT[:, c * NB:(c + 1) * NB]),
            "vT": vT,
            "keep": np.ascontiguousarray(keep[c * NB:(c + 1) * NB]),
            "idxs": np.ascontiguousarray(idxs[c * NB:(c + 1) * NB]),
        }
        for c in range(N_CORES)
    ]
    return in_maps, widths


def gather_output(results):
    out = np.empty((B, S), dtype=np.float32)
    for c in range(N_CORES):
        out[c * NB:(c + 1) * NB] = results[c]["out"].reshape(NB, S)
    return out


def kernel(memory, target, memory_mask, Wq, Wk, v):
    from concourse.bass_utils import run_bass_kernel_spmd

    in_maps, widths = prepare_in_maps(memory, target, memory_mask, Wq, Wk, v)
    nc = get_program(widths=widths)
    res = run_bass_kernel_spmd(nc, in_maps, list(range(N_CORES)))
    return gather_output(res.results)



# revision 5
# speedup vs baseline: 1.3949x; 1.3949x over previous
"""Additive (Bahdanau) attention scoring kernel for Trainium2, 8-core SPMD.

Reference computation (B=16, S=4096, D=1024, all fp32):
    q      = target @ Wq.T                    # [B, D]
    k      = memory @ Wk.T                    # [B, S, D]
    scores = tanh(q[:, None, :] + k) @ v      # [B, S]
    out    = softmax(scores - 1e9 * mask, axis=-1)

Sharding: batch across the 8 cores (2 batches per core), weights replicated.

Host-side prep (layout only): memory is transposed to [D, S] per batch and
its columns compacted to just the unmasked positions (padded with duplicates
of the first kept column to a 128-multiple). Masked positions contribute
exactly 0 to the reference softmax (exp(-1e9) == 0 in fp32), so skipping
their columns is algebraically exact. The kernel emits the compact softmax
rows; the host scatters them back to full-S positions (pad columns are
masked out on device before the softmax sum, so they never contribute).

Device pipeline ([s, e] layout — mem chunks stationary, Wk moving, bf16):
  - All matmul operands are cast to bf16 on device; accumulation stays fp32
    in PSUM. Max-rel-err impact ~3e-3 (sim-verified), under the 2e-2 gate.
  - k^T block [s=128, e=1024] = sum_dc memb[dc].T @ wkb[dc] — 16 MMs of
    N=512 per block into two PSUM banks; the stationary operand is the
    streamed memory chunk, the moving operand is the resident Wk. bf16
    enables fast-weight-load, so LDWEIGHTS hides behind the MM stream.
  - q is computed once per batch ([1, 512] psum rows via target-stationary
    MMs emitted mid-first-sgroup so the PE never waits on the Wq DMA),
    then broadcast to [128, D] with a K=1 ones-matmul (partition_broadcast
    returned stale data on HW; tensor_tensor_reduce faulted the device —
    both are avoided deliberately).
  - Per block: DVE adds q to the k psum (q varies along the free dim, so
    ACT's per-partition bias can't), ACT applies tanh, DVE does the v-dot
    as tensor_mul + reduce_sum into a per-partition scores column. GpSimd
    does the fp32->bf16 mem casts. No PE v-dot, no exp-strip DRAM bounces,
    no indirect-DMA scatter.
  - Finale per batch: exp on ACT, keep-mask multiply + row-sum on DVE,
    ones-matmul partition total, reciprocal, scale, compact DMA out.
"""

import numpy as np

from contextlib import ExitStack

import concourse.tile as tile
from concourse import bacc, mybir

B, S, D = 16, 4096, 1024
N_CORES = 8
NB = B // N_CORES  # batches per core
P = 128
DC = D // P        # contraction chunks (8)
SW = 512           # sgroup width along compacted s

F32 = mybir.dt.float32
BF16 = mybir.dt.bfloat16
AF = mybir.ActivationFunctionType
ALU = mybir.AluOpType

_CACHE = {}


def strip_widths(max_kept):
    """Sgroup widths covering max_kept compacted columns: full 512-wide
    groups plus a 128-granular tail of at least 256."""
    total = max(512, ((max_kept + 127) // 128) * 128)
    widths = [SW] * (total // SW)
    rem = total % SW
    if rem:
        widths.append(max(256, rem))
    return tuple(widths)


def _build_program(widths):
    s_pad = sum(widths)
    nblk = s_pad // P  # score columns per batch

    nc = bacc.Bacc("TRN2", target_bir_lowering=False, debug=False)

    memC = nc.dram_tensor("memC", [NB, DC * P, s_pad], F32, kind="ExternalInput").ap()
    wkT = nc.dram_tensor("wkT", [DC * P, D], F32, kind="ExternalInput").ap()
    wqT = nc.dram_tensor("wqT", [DC * P, D], F32, kind="ExternalInput").ap()
    tgtT = nc.dram_tensor("tgtT", [D, NB], F32, kind="ExternalInput").ap()
    vrow = nc.dram_tensor("vrow", [1, D], F32, kind="ExternalInput").ap()
    keepC = nc.dram_tensor("keepC", [NB, P, nblk], F32, kind="ExternalInput").ap()
    out = nc.dram_tensor("out", [NB, P, nblk], F32, kind="ExternalOutput").ap()

    with tile.TileContext(nc) as tc, ExitStack() as ctx:
        consts = ctx.enter_context(tc.tile_pool(name="consts", bufs=1))
        wst_pool = ctx.enter_context(tc.tile_pool(name="wst", bufs=2))
        mst_pool = ctx.enter_context(tc.tile_pool(name="mst", bufs=2))
        mb_pool = ctx.enter_context(tc.tile_pool(name="mb", bufs=2))
        work_pool = ctx.enter_context(tc.tile_pool(name="work", bufs=3))
        fin_pool = ctx.enter_context(tc.tile_pool(name="fin", bufs=2))
        kps_pool = ctx.enter_context(tc.tile_pool(name="kps", bufs=6, space="PSUM"))
        sm_pool = ctx.enter_context(tc.tile_pool(name="smps", bufs=2, space="PSUM"))

        # --- small constants ---
        tgt_sb = consts.tile([P, DC * NB], F32)
        for dc in range(DC):
            nc.sync.dma_start(tgt_sb[:, dc * NB:(dc + 1) * NB], tgtT[dc * P:(dc + 1) * P, :])
        tgt16 = consts.tile([P, DC * NB], BF16)
        nc.vector.tensor_copy(tgt16[:], tgt_sb[:])
        v_sb = consts.tile([1, D], F32)
        nc.sync.dma_start(v_sb[:], vrow[:, :])
        v16 = consts.tile([1, D], BF16)
        nc.vector.tensor_copy(v16[:], v_sb[:])
        keep_sb = consts.tile([P, NB * nblk], F32)
        for b in range(NB):
            nc.sync.dma_start(keep_sb[:, b * nblk:(b + 1) * nblk], keepC[b])
        ones1 = consts.tile([1, P], BF16)
        nc.vector.memset(ones1[:], 1.0)
        ones_sb = consts.tile([P, P], F32)
        nc.vector.memset(ones_sb[:], 1.0)

        # v broadcast [P, D] bf16 via K=1 ones-matmul + PSUM->SBUF copy
        v_bcast = consts.tile([P, D], BF16)
        for eh in range(2):
            vb_ps = sm_pool.tile([P, SW], F32, tag="small", name="vb_ps")
            nc.tensor.matmul(vb_ps[:], ones1[:], v16[:, eh * SW:(eh + 1) * SW],
                             start=True, stop=True)
            nc.vector.tensor_copy(v_bcast[:, eh * SW:(eh + 1) * SW], vb_ps[:])

        # --- Wk chunk-by-chunk (the k-matmuls need it first; Wq DMAs are
        # issued inside emit_q, after the first sgroup's mem DMAs) ---
        wkb = consts.tile([P, DC * D], BF16)
        for dc in range(DC):
            wk_st = wst_pool.tile([P, D], F32, tag="wst", name="wk_st")
            nc.sync.dma_start(wk_st[:], wkT[dc * P:(dc + 1) * P, :])
            nc.scalar.activation(wkb[:, dc * D:(dc + 1) * D], wk_st[:], AF.Copy)

        q_bcast = [consts.tile([P, D], F32, tag=f"qb{b}", name=f"qb{b}") for b in range(NB)]
        q_rows = [consts.tile([1, D], BF16, tag=f"qr{b}", name=f"qr{b}") for b in range(NB)]

        def emit_q():
            # q[b, e] = sum_d target[b, d] * Wq[e, d]: per-batch [1, 512]
            # psum rows (stationary = one target column), then a K=1
            # ones-matmul broadcast to [128, D].
            wqb = consts.tile([P, DC * D], BF16)
            for dc in range(DC):
                wq_st = wst_pool.tile([P, D], F32, tag="wst", name="wq_st")
                nc.sync.dma_start(wq_st[:], wqT[dc * P:(dc + 1) * P, :])
                nc.scalar.activation(wqb[:, dc * D:(dc + 1) * D], wq_st[:], AF.Copy)
            for b in range(NB):
                for eh in range(2):
                    q_ps = sm_pool.tile([1, SW], F32, tag="small", name="q_ps")
                    for dc in range(DC):
                        nc.tensor.matmul(
                            q_ps[:],
                            tgt16[:, dc * NB + b: dc * NB + b + 1],
                            wqb[:, dc * D + eh * SW: dc * D + (eh + 1) * SW],
                            start=(dc == 0),
                            stop=(dc == DC - 1),
                        )
                    nc.vector.tensor_copy(q_rows[b][:, eh * SW:(eh + 1) * SW], q_ps[:])
            for b in range(NB):
                for eh in range(2):
                    qb_ps = sm_pool.tile([P, SW], F32, tag="small", name="qb_ps")
                    nc.tensor.matmul(qb_ps[:], ones1[:],
                                     q_rows[b][:, eh * SW:(eh + 1) * SW],
                                     start=True, stop=True)
                    nc.vector.tensor_copy(q_bcast[b][:, eh * SW:(eh + 1) * SW], qb_ps[:])

        scores = [fin_pool.tile([P, nblk], F32, tag=f"sc{b}", name=f"sc{b}") for b in range(NB)]

        def emit_mem_load(b, w, off):
            mem_st = mst_pool.tile([P, DC * SW], F32, tag="mem_st", name="mem_st")
            for dc in range(DC):
                nc.sync.dma_start(
                    mem_st[:, dc * SW:dc * SW + w],
                    memC[b, dc * P:(dc + 1) * P, off:off + w],
                )
            memb = mb_pool.tile([P, DC * SW], BF16, tag="memb", name="memb")
            if w == SW:
                nc.gpsimd.tensor_copy(memb[:], mem_st[:])
            else:
                for dc in range(DC):
                    nc.gpsimd.tensor_copy(
                        memb[:, dc * SW:dc * SW + w], mem_st[:, dc * SW:dc * SW + w]
                    )
            return memb

        def emit_block_mms(memb, j):
            k_ps = [kps_pool.tile([P, SW], F32, tag="k", name="k_ps") for _ in range(2)]
            for dc in range(DC):
                for eh in range(2):
                    nc.tensor.matmul(
                        k_ps[eh][:],
                        memb[:, dc * SW + j * P: dc * SW + (j + 1) * P],
                        wkb[:, dc * D + eh * SW: dc * D + (eh + 1) * SW],
                        start=(dc == 0),
                        stop=(dc == DC - 1),
                    )
            return k_ps

        def emit_block_post(b, k_ps, jcol):
            tt = work_pool.tile([P, D], BF16, tag="tt", name="tt")
            for eh in range(2):
                nc.vector.tensor_add(
                    tt[:, eh * SW:(eh + 1) * SW], k_ps[eh][:],
                    q_bcast[b][:, eh * SW:(eh + 1) * SW],
                )
            th = work_pool.tile([P, D], BF16, tag="th", name="th")
            nc.scalar.activation(th[:], tt[:], AF.Tanh)
            prod = work_pool.tile([P, D], BF16, tag="prod", name="prod")
            nc.vector.tensor_mul(prod[:], th[:], v_bcast[:])
            nc.vector.reduce_sum(scores[b][:, jcol:jcol + 1], prod[:],
                                 axis=mybir.AxisListType.X)

        first = True
        for b in range(NB):
            off = 0
            for sp, w in enumerate(widths):
                nb_w = w // P
                memb = emit_mem_load(b, w, off)
                if first:
                    # First sgroup: the PE starts on k-matmuls as soon as
                    # Wk+mem land, with the q chain emitted before the PE
                    # can run out of PSUM slots (blocks 2+ reuse slots that
                    # are only freed by DVE adds, which need q_bcast — so q
                    # must sit ahead of them in the PE FIFO), and before the
                    # first DVE add (the q_row copies must precede their
                    # consumers in the DVE FIFO).
                    k_blocks = [emit_block_mms(memb, j) for j in range(min(2, nb_w))]
                    emit_q()
                    for j in range(2, nb_w):
                        k_blocks.append(emit_block_mms(memb, j))
                    for j in range(nb_w):
                        emit_block_post(b, k_blocks[j], off // P + j)
                    first = False
                else:
                    for j in range(nb_w):
                        k_ps = emit_block_mms(memb, j)
                        emit_block_post(b, k_ps, off // P + j)
                off += w

        # --- masked softmax finale per batch (compact layout) ---
        for b in range(NB):
            esq = fin_pool.tile([P, nblk], F32, tag="esq", name="esq")
            nc.scalar.activation(esq[:], scores[b][:], AF.Exp)
            em = fin_pool.tile([P, nblk], F32, tag="em", name="em")
            part = fin_pool.tile([P, 1], F32, tag="part", name="part")
            nc.vector.tensor_mul(em[:], esq[:], keep_sb[:, b * nblk:(b + 1) * nblk])
            nc.vector.reduce_sum(part[:], em[:], axis=mybir.AxisListType.X)
            tot_ps = sm_pool.tile([P, 1], F32, tag="small", name="tot_ps")
            nc.tensor.matmul(tot_ps[:], ones_sb[:], part[:], start=True, stop=True)
            recip = fin_pool.tile([P, 1], F32, tag="recip", name="recip")
            nc.vector.reciprocal(recip[:], tot_ps[:])
            outt = fin_pool.tile([P, nblk], F32, tag="outt", name="outt")
            nc.vector.tensor_scalar_mul(outt[:], em[:], recip[:, 0:1])
            nc.sync.dma_start(out[b], outt[:])

    nc.compile()
    return nc


def get_program(widths=None):
    assert widths is not None
    if widths not in _CACHE:
        _CACHE[widths] = _build_program(widths)
    return _CACHE[widths]


def prepare_in_maps(memory, target, memory_mask, Wq, Wk, v):
    memory = np.asarray(memory, dtype=np.float32)
    target = np.asarray(target, dtype=np.float32)
    Wq = np.asarray(Wq, dtype=np.float32)
    Wk = np.asarray(Wk, dtype=np.float32)
    v = np.asarray(v, dtype=np.float32)
    mask = np.asarray(memory_mask)

    # host-side sharding / layout prep (no arithmetic)
    keep_bool = ~mask                                                # [B, S]
    widths = strip_widths(int(keep_bool.sum(1).max()))
    s_pad = sum(widths)
    nblk = s_pad // P

    memT = memory.transpose(0, 2, 1)                                 # [B, D, S] view
    kept_pad = np.empty((B, s_pad), dtype=np.int64)
    kept_count = np.empty(B, dtype=np.int64)
    for b in range(B):
        k = np.flatnonzero(keep_bool[b])
        kept_count[b] = len(k)
        kept_pad[b, :len(k)] = k
        kept_pad[b, len(k):] = k[0]  # pad data: duplicate first kept column
    memC = np.empty((B, D, s_pad), dtype=np.float32)
    for b in range(B):
        memC[b] = memT[b][:, kept_pad[b]]

    # compact keep mask in [128, nblk] layout: compact position j*128+p -> [p, j]
    keepC = np.zeros((B, s_pad), dtype=np.float32)
    for b in range(B):
        keepC[b, :kept_count[b]] = 1.0
    keepC = np.ascontiguousarray(keepC.reshape(B, nblk, P).transpose(0, 2, 1))

    wkT = np.ascontiguousarray(Wk.T)                                 # [D, D]
    wqT = np.ascontiguousarray(Wq.T)                                 # [D, D]
    tgtT = np.ascontiguousarray(target.T)                            # [D, B]
    vr = np.ascontiguousarray(v.reshape(1, D))                       # [1, D]

    in_maps = [
        {
            "memC": np.ascontiguousarray(memC[c * NB:(c + 1) * NB]),
            "wkT": wkT,
            "wqT": wqT,
            "tgtT": np.ascontiguousarray(tgtT[:, c * NB:(c + 1) * NB]),
            "vrow": vr,
            "keepC": np.ascontiguousarray(keepC[c * NB:(c + 1) * NB]),
        }
        for c in range(N_CORES)
    ]
    global _LAST_META
    _LAST_META = (kept_pad, kept_count, nblk)
    return in_maps, widths


_LAST_META = None


def gather_output(results, meta=None):
    kept_pad, kept_count, nblk = meta if meta is not None else _LAST_META
    out = np.zeros((B, S), dtype=np.float32)
    for c in range(N_CORES):
        vals = results[c]["out"].reshape(NB, P, nblk)                # [b, p, j]
        for bb in range(NB):
            b = c * NB + bb
            compact = vals[bb].T.reshape(-1)                         # compact position j*128+p
            kc = kept_count[b]
            out[b, kept_pad[b, :kc]] = compact[:kc]
    return out


def kernel(memory, target, memory_mask, Wq, Wk, v):
    from concourse.bass_utils import run_bass_kernel_spmd

    in_maps, widths = prepare_in_maps(memory, target, memory_mask, Wq, Wk, v)
    nc = get_program(widths=widths)
    res = run_bass_kernel_spmd(nc, in_maps, list(range(N_CORES)))
    return gather_output(res.results)


# revision 7
# speedup vs baseline: 1.6815x; 1.2055x over previous
"""Additive (Bahdanau) attention scoring kernel for Trainium2, 8-core SPMD.

Reference computation (B=16, S=4096, D=1024, all fp32):
    q      = target @ Wq.T                    # [B, D]
    k      = memory @ Wk.T                    # [B, S, D]
    scores = tanh(q[:, None, :] + k) @ v      # [B, S]
    out    = softmax(scores - 1e9 * mask, axis=-1)

Sharding: batch across the 8 cores (2 batches per core), weights replicated.

Host-side prep: memory is transposed to [D, S] per batch and its columns
compacted to just the unmasked positions (padded with duplicates of the
first kept column to a 128-multiple). Masked positions contribute exactly 0
to the reference softmax (exp(-1e9) == 0 in fp32), so skipping their
columns is algebraically exact. Large operands are shipped in bf16 — the
kernel's internal matmul precision (max rel err ~3e-3 vs the 2e-2 gate).
The kernel emits compact softmax rows; the host scatters them back to
full-S positions (pad columns get -1e9 added to their scores on device, so
they contribute exp(-1e9)=0 to the softmax sum and are then discarded).

Device pipeline ([e, s] layout, everything on the PE + ACT):
  - k^T tile [e=128, s<=512] accumulates over dc: stationary = WkT chunk
    [128, 128], moving = resident mem batch slice [128, w]. dc-outer over
    half the e-tiles (4 PSUM banks) so the PE tracks the weight/mem DMAs
    during the prologue instead of stalling on the full set.
  - q^T via target-stationary MMs into [2, 512] psum, DRAM bounce into
    per-partition bias layout [128, 2*8] — then the q-add is FREE inside
    the ACT tanh (per-partition bias), writing bf16.
  - v-dot stays on the PE: vd[1, w] += vT[:, et].T @ tanh_tile, plus one
    K=1 matmul adding the compact -1e9 pad mask row; ACT Exp reads the
    psum strip with accum_out producing the softmax partial sum for free.
  - Finale per batch: DVE reduce of the strip partials + reciprocal, one
    ACT Copy(scale=1/sum) over the whole compact row, single-descriptor
    DMA out. DVE/GpSimd are otherwise idle; no casts, no scatter.
"""

import numpy as np
import ml_dtypes

from contextlib import ExitStack

import concourse.tile as tile
from concourse import bacc, mybir

B, S, D = 16, 4096, 1024
N_CORES = 8
NB = B // N_CORES  # batches per core
P = 128
DC = D // P        # contraction chunks (8)
ET = D // P        # e tiles (8)
SW = 512           # substrip width (PSUM bank limit at fp32)

F32 = mybir.dt.float32
BF16 = mybir.dt.bfloat16
AF = mybir.ActivationFunctionType

BF16NP = ml_dtypes.bfloat16

_CACHE = {}


def substrips(s_pad):
    widths = [SW] * (s_pad // SW)
    if s_pad % SW:
        widths.append(s_pad % SW)
    return widths


def _build_program(s_pad):
    widths = substrips(s_pad)
    nsub = len(widths)

    nc = bacc.Bacc("TRN2", target_bir_lowering=False, debug=False)

    memC = nc.dram_tensor("memC", [NB, DC * P, s_pad], BF16, kind="ExternalInput").ap()
    wkT = nc.dram_tensor("wkT", [DC * P, D], BF16, kind="ExternalInput").ap()
    wqT = nc.dram_tensor("wqT", [DC * P, D], BF16, kind="ExternalInput").ap()
    tgtT = nc.dram_tensor("tgtT", [D, NB], BF16, kind="ExternalInput").ap()
    vT = nc.dram_tensor("vT", [P, ET], BF16, kind="ExternalInput").ap()
    mneg = nc.dram_tensor("mneg", [NB, s_pad], BF16, kind="ExternalInput").ap()
    out = nc.dram_tensor("out", [NB, s_pad], F32, kind="ExternalOutput").ap()

    with tile.TileContext(nc) as tc, ExitStack() as ctx:
        consts = ctx.enter_context(tc.tile_pool(name="consts", bufs=1))
        mb_pool = ctx.enter_context(tc.tile_pool(name="mb", bufs=2))
        th_pool = ctx.enter_context(tc.tile_pool(name="th", bufs=4))
        fin_pool = ctx.enter_context(tc.tile_pool(name="fin", bufs=2))
        kps_pool = ctx.enter_context(tc.tile_pool(name="kps", bufs=4, space="PSUM"))
        vd_pool = ctx.enter_context(tc.tile_pool(name="vd", bufs=2, space="PSUM"))
        sm_pool = ctx.enter_context(tc.tile_pool(name="smps", bufs=2, space="PSUM"))
        dram_pool = ctx.enter_context(tc.tile_pool(name="scratch", bufs=1, space="DRAM"))

        # --- tiny consts ---
        tgt16 = consts.tile([P, DC * NB], BF16)
        for dc in range(DC):
            nc.sync.dma_start(tgt16[:, dc * NB:(dc + 1) * NB], tgtT[dc * P:(dc + 1) * P, :])
        v_sb = consts.tile([P, ET], BF16)
        nc.sync.dma_start(v_sb[:], vT[:, :])
        mneg_sb = consts.tile([1, NB * s_pad], BF16)
        for b in range(NB):
            nc.sync.dma_start(mneg_sb[:, b * s_pad:(b + 1) * s_pad], mneg[b:b + 1, :])
        one11 = consts.tile([1, 1], BF16)
        nc.vector.memset(one11[:], 1.0)

        # --- weights + batch-0 substrip-0 mem interleaved (critical path to
        # the first k-matmul), then Wq, then the rest of the memory ---
        wkb = consts.tile([P, DC * D], BF16)
        membs = [mb_pool.tile([P, DC * s_pad], BF16, tag="memb", name=f"memb{b}")
                 for b in range(NB)]
        w0 = widths[0]
        for dc in range(DC):
            nc.sync.dma_start(wkb[:, dc * D:(dc + 1) * D], wkT[dc * P:(dc + 1) * P, :])
            nc.sync.dma_start(
                membs[0][:, dc * s_pad: dc * s_pad + w0],
                memC[0, dc * P:(dc + 1) * P, 0:w0],
            )
        wqb = consts.tile([P, DC * D], BF16)
        for dc in range(DC):
            nc.sync.dma_start(wqb[:, dc * D:(dc + 1) * D], wqT[dc * P:(dc + 1) * P, :])
        # rest of batch 0's memory (per-dc, per-substrip chunks so early
        # substrips land early), then batch 1 whole rows
        for sp in range(1, nsub):
            off = sum(widths[:sp])
            w = widths[sp]
            for dc in range(DC):
                nc.sync.dma_start(
                    membs[0][:, dc * s_pad + off: dc * s_pad + off + w],
                    memC[0, dc * P:(dc + 1) * P, off:off + w],
                )
        for dc in range(DC):
            nc.sync.dma_start(membs[1][:, dc * s_pad:(dc + 1) * s_pad],
                              memC[1, dc * P:(dc + 1) * P, :])

        q_sb = consts.tile([P, NB * ET], F32)
        scores = [consts.tile([1, s_pad], F32, tag=f"str{b}", name=f"str{b}")
                  for b in range(NB)]
        accs = [consts.tile([1, nsub], F32, tag=f"acc{b}", name=f"acc{b}")
                for b in range(NB)]

        def emit_q():
            # q[b, e] = sum_d target[b, d] * Wq[e, d]: [2, 512] psum halves
            # (M=2-stationary keeps it to 16 big MMs), bounced through DRAM
            # into per-partition bias layout [128, NB*ET].
            q2 = consts.tile([NB, D], F32)
            for eh in range(2):
                q_ps = sm_pool.tile([NB, SW], F32, tag="small", name="q_ps")
                for dc in range(DC):
                    nc.tensor.matmul(
                        q_ps[:],
                        tgt16[:, dc * NB:(dc + 1) * NB],
                        wqb[:, dc * D + eh * SW: dc * D + (eh + 1) * SW],
                        start=(dc == 0),
                        stop=(dc == DC - 1),
                    )
                nc.vector.tensor_copy(q2[:, eh * SW:(eh + 1) * SW], q_ps[:])
            qscr = dram_pool.tile([NB, D], F32, tag="qscr", name="qscr")
            nc.sync.dma_start(qscr[:], q2[:])
            for b in range(NB):
                nc.sync.dma_start(
                    q_sb[:, b * ET:(b + 1) * ET],
                    qscr[b].rearrange("(et p) -> p et", p=P),
                )

        first = True
        for b in range(NB):
            off = 0
            for sp, w in enumerate(widths):
                vd_ps = vd_pool.tile([1, SW], F32, tag="vd", name="vd_ps")
                ths = {}
                for half in range(2):
                    ets = range(half * 4, half * 4 + 4)
                    k_ps = {et: kps_pool.tile([P, SW], F32, tag="k", name="k_ps")
                            for et in ets}
                    for dc in range(DC):
                        for et in ets:
                            nc.tensor.matmul(
                                k_ps[et][:, :w],
                                wkb[:, dc * D + et * P: dc * D + (et + 1) * P],
                                membs[b][:, dc * s_pad + off: dc * s_pad + off + w],
                                start=(dc == 0),
                                stop=(dc == DC - 1),
                            )
                    if first:
                        # q rides after substrip 0's first half: the PE
                        # reaches it while Wq is still landing, and the
                        # tanh bias is ready before the first ACT tanh.
                        emit_q()
                        first = False
                    for et in ets:
                        th = th_pool.tile([P, SW], BF16, tag="th", name="th")
                        nc.scalar.activation(
                            th[:, :w], k_ps[et][:, :w], AF.Tanh,
                            bias=q_sb[:, b * ET + et: b * ET + et + 1],
                        )
                        ths[et] = th
                        # v-dot trails the tanh stream on the PE
                        if et >= 2:
                            lag = et - 2
                            nc.tensor.matmul(
                                vd_ps[:, :w], v_sb[:, lag:lag + 1], ths[lag][:, :w],
                                start=(lag == 0), stop=False,
                            )
                for et in (ET - 2, ET - 1):
                    nc.tensor.matmul(
                        vd_ps[:, :w], v_sb[:, et:et + 1], ths[et][:, :w],
                        start=False, stop=False,
                    )
                nc.tensor.matmul(
                    vd_ps[:, :w], one11[:],
                    mneg_sb[:, b * s_pad + off: b * s_pad + off + w],
                    start=False, stop=True,
                )
                nc.scalar.activation(
                    scores[b][:, off:off + w], vd_ps[:, :w], AF.Exp,
                    accum_out=accs[b][:, sp:sp + 1],
                )
                off += w

        # --- softmax normalization per batch (compact row) ---
        for b in range(NB):
            tot = fin_pool.tile([1, 1], F32, tag="tot", name="tot")
            nc.vector.reduce_sum(tot[:], accs[b][:], axis=mybir.AxisListType.X)
            recip = fin_pool.tile([1, 1], F32, tag="recip", name="recip")
            nc.vector.reciprocal(recip[:], tot[:])
            outs = fin_pool.tile([1, s_pad], F32, tag="outs", name="outs")
            nc.scalar.activation(outs[:], scores[b][:], AF.Copy, scale=recip[:, 0:1])
            nc.sync.dma_start(out[b:b + 1, :], outs[:])

    nc.compile()
    return nc


def get_program(s_pad=None):
    assert s_pad is not None
    if s_pad not in _CACHE:
        _CACHE[s_pad] = _build_program(s_pad)
    return _CACHE[s_pad]


def prepare_in_maps(memory, target, memory_mask, Wq, Wk, v):
    memory = np.asarray(memory, dtype=np.float32)
    target = np.asarray(target, dtype=np.float32)
    Wq = np.asarray(Wq, dtype=np.float32)
    Wk = np.asarray(Wk, dtype=np.float32)
    v = np.asarray(v, dtype=np.float32)
    mask = np.asarray(memory_mask)

    keep_bool = ~mask                                                # [B, S]
    max_kept = int(keep_bool.sum(1).max())
    s_pad = max(512, ((max_kept + 127) // 128) * 128)

    memT = memory.transpose(0, 2, 1)                                 # [B, D, S] view
    kept_pad = np.empty((B, s_pad), dtype=np.int64)
    kept_count = np.empty(B, dtype=np.int64)
    for b in range(B):
        k = np.flatnonzero(keep_bool[b])
        kept_count[b] = len(k)
        kept_pad[b, :len(k)] = k
        kept_pad[b, len(k):] = k[0]  # pad data: duplicate first kept column
    memC = np.empty((B, D, s_pad), dtype=BF16NP)
    for b in range(B):
        memC[b] = memT[b][:, kept_pad[b]]

    # compact pad mask: 0 at kept positions, -1e9 at pads (pads then produce
    # exp(-1e9) == 0 and never pollute the softmax sum)
    mnegC = np.zeros((B, s_pad), dtype=np.float32)
    for b in range(B):
        mnegC[b, kept_count[b]:] = -1e9
    mnegC = mnegC.astype(BF16NP)

    wkT = np.ascontiguousarray(Wk.T).astype(BF16NP)                  # [D, D]
    wqT = np.ascontiguousarray(Wq.T).astype(BF16NP)                  # [D, D]
    tgtT = np.ascontiguousarray(target.T).astype(BF16NP)             # [D, B]
    vTh = np.ascontiguousarray(v.reshape(ET, P).T).astype(BF16NP)    # [P, ET]

    in_maps = [
        {
            "memC": np.ascontiguousarray(memC[c * NB:(c + 1) * NB]),
            "wkT": wkT,
            "wqT": wqT,
            "tgtT": np.ascontiguousarray(tgtT[:, c * NB:(c + 1) * NB]),
            "vT": vTh,
            "mneg": np.ascontiguousarray(mnegC[c * NB:(c + 1) * NB]),
        }
        for c in range(N_CORES)
    ]
    global _LAST_META
    _LAST_META = (kept_pad, kept_count, s_pad)
    return in_maps, s_pad


_LAST_META = None


def gather_output(results, meta=None):
    kept_pad, kept_count, s_pad = meta if meta is not None else _LAST_META
    out = np.zeros((B, S), dtype=np.float32)
    for c in range(N_CORES):
        vals = results[c]["out"].reshape(NB, s_pad)
        for bb in range(NB):
            b = c * NB + bb
            kc = kept_count[b]
            out[b, kept_pad[b, :kc]] = vals[bb, :kc]
    return out


def kernel(memory, target, memory_mask, Wq, Wk, v):
    from concourse.bass_utils import run_bass_kernel_spmd

    in_maps, s_pad = prepare_in_maps(memory, target, memory_mask, Wq, Wk, v)
    nc = get_program(s_pad=s_pad)
    res = run_bass_kernel_spmd(nc, in_maps, list(range(N_CORES)))
    return gather_output(res.results)


# revision 9
# speedup vs baseline: 1.9422x; 1.1550x over previous
"""Additive (Bahdanau) attention scoring kernel for Trainium2, 8-core SPMD.

Reference computation (B=16, S=4096, D=1024, all fp32):
    q      = target @ Wq.T                    # [B, D]
    k      = memory @ Wk.T                    # [B, S, D]
    scores = tanh(q[:, None, :] + k) @ v      # [B, S]
    out    = softmax(scores - 1e9 * mask, axis=-1)

Sharding: batch across the 8 cores (2 batches per core), weights replicated.

Host-side prep: memory is transposed and its columns compacted to just the
unmasked positions (padded with duplicates of the first kept column to a
128-multiple). Masked positions contribute exactly 0 to the reference
softmax (exp(-1e9) == 0 in fp32), so skipping their columns is exact.
Large operands ship in bf16 — the kernel's internal matmul precision (max
rel err ~3e-3 vs the 2e-2 gate). memC is partition-major ([P, DC, s]) so a
whole batch loads with ONE dma_start of 128 x 34.8KB descriptors — the
sync-sequencer's ~0.6us-per-issue cost was the previous bottleneck.
The kernel emits compact softmax rows; the host scatters them back to
full-S positions (pad columns get -1e9 added on device, so they contribute
exp(-1e9)=0 to the softmax sum and are then discarded).

Device pipeline ([e, s] layout, everything on the PE + ACT):
  - k^T tile [e=128, s<=512] accumulates over dc: stationary = WkT chunk,
    moving = resident mem batch slice. dc-outer over half the e-tiles
    (4 PSUM banks) so the PE tracks the weight DMAs during the prologue.
  - q^T is computed directly in bias layout: per e-tile [128, 2] psum via
    Wq-chunk-stationary x target moving (64 tiny MMs, no DRAM bounce);
    the q-add is then FREE inside the ACT tanh (per-partition bias).
  - v-dot on the PE at full width: stationary V_et = v-chunk broadcast
    across 128 columns (every output row equals the v-dot), moving = tanh
    tile; skinny M=1 matmuls measured 306ns vs 216ns full-width. The pad
    mask row is added by one K=1 matmul into the same PSUM group; ACT Exp
    reads psum row 0 with accum_out producing the softmax sum for free.
  - Finale per batch: DVE reduce + reciprocal, one ACT Copy(scale=1/sum)
    over the compact row, single-descriptor DMA out.
"""

import numpy as np
import ml_dtypes

from contextlib import ExitStack

import concourse.tile as tile
from concourse import bacc, mybir

B, S, D = 16, 4096, 1024
N_CORES = 8
NB = B // N_CORES  # batches per core
P = 128
DC = D // P        # contraction chunks (8)
ET = D // P        # e tiles (8)
SW = 512           # substrip width (PSUM bank limit at fp32)

F32 = mybir.dt.float32
BF16 = mybir.dt.bfloat16
AF = mybir.ActivationFunctionType

BF16NP = ml_dtypes.bfloat16

_CACHE = {}


def substrips(s_pad):
    widths = [SW] * (s_pad // SW)
    if s_pad % SW:
        widths.append(s_pad % SW)
    return widths


def _build_program(s_pad):
    widths = substrips(s_pad)
    nsub = len(widths)

    nc = bacc.Bacc("TRN2", target_bir_lowering=False, debug=False)

    memC = nc.dram_tensor("memC", [NB, P, DC, s_pad], BF16, kind="ExternalInput").ap()
    wkT = nc.dram_tensor("wkT", [DC * P, D], BF16, kind="ExternalInput").ap()
    wqT = nc.dram_tensor("wqT", [DC * P, D], BF16, kind="ExternalInput").ap()
    tgtT = nc.dram_tensor("tgtT", [D, NB], BF16, kind="ExternalInput").ap()
    vT = nc.dram_tensor("vT", [P, ET], F32, kind="ExternalInput").ap()
    mneg = nc.dram_tensor("mneg", [NB, s_pad], BF16, kind="ExternalInput").ap()
    out = nc.dram_tensor("out", [NB, s_pad], F32, kind="ExternalOutput").ap()

    with tile.TileContext(nc) as tc, ExitStack() as ctx:
        consts = ctx.enter_context(tc.tile_pool(name="consts", bufs=1))
        mb_pool = ctx.enter_context(tc.tile_pool(name="mb", bufs=2))
        th_pool = ctx.enter_context(tc.tile_pool(name="th", bufs=4))
        fin_pool = ctx.enter_context(tc.tile_pool(name="fin", bufs=2))
        kps_pool = ctx.enter_context(tc.tile_pool(name="kps", bufs=4, space="PSUM"))
        vd_pool = ctx.enter_context(tc.tile_pool(name="vd", bufs=2, space="PSUM"))
        sm_pool = ctx.enter_context(tc.tile_pool(name="smps", bufs=2, space="PSUM"))

        # --- DMA issue order is the prologue critical path: two Wk chunks,
        # then batch-0 substrip 0, then the rest of Wk, the rest of batch 0,
        # Wq, batch 1, and only then the tiny consts. ---
        wkb = consts.tile([P, DC * D], BF16)
        membs = [mb_pool.tile([P, DC, s_pad], BF16, tag="memb", name=f"memb{b}")
                 for b in range(NB)]
        w0 = widths[0]
        for dc in range(2):
            nc.sync.dma_start(wkb[:, dc * D:(dc + 1) * D], wkT[dc * P:(dc + 1) * P, :])
        nc.sync.dma_start(membs[0][:, :, 0:w0], memC[0, :, :, 0:w0])
        for dc in range(2, DC):
            nc.sync.dma_start(wkb[:, dc * D:(dc + 1) * D], wkT[dc * P:(dc + 1) * P, :])
        nc.sync.dma_start(membs[0][:, :, w0:s_pad], memC[0, :, :, w0:s_pad])
        wqb = consts.tile([P, DC * D], BF16)
        for dc in range(DC):
            nc.sync.dma_start(wqb[:, dc * D:(dc + 1) * D], wqT[dc * P:(dc + 1) * P, :])
        nc.sync.dma_start(membs[1][:, :, :], memC[1, :, :, :])

        tgt16 = consts.tile([P, DC * NB], BF16)
        for dc in range(DC):
            nc.sync.dma_start(tgt16[:, dc * NB:(dc + 1) * NB], tgtT[dc * P:(dc + 1) * P, :])
        v_sb = consts.tile([P, ET], F32)
        nc.sync.dma_start(v_sb[:], vT[:, :])
        mneg_sb = consts.tile([1, NB * s_pad], BF16)
        for b in range(NB):
            nc.sync.dma_start(mneg_sb[:, b * s_pad:(b + 1) * s_pad], mneg[b:b + 1, :])
        one1p = consts.tile([1, P], BF16)
        nc.vector.memset(one1p[:], 1.0)
        ones128 = consts.tile([P, P], BF16)
        nc.vector.memset(ones128[:], 1.0)
        # V_et = v chunk broadcast across 128 columns (per-partition scalar
        # broadcast along the free dim — a native DVE tensor_scalar op)
        V_all = consts.tile([P, ET * P], BF16)
        for et in range(ET):
            nc.vector.tensor_scalar_mul(V_all[:, et * P:(et + 1) * P], ones128[:],
                                        v_sb[:, et:et + 1])

        q_sb = consts.tile([P, NB * ET], F32)
        scores = [consts.tile([1, s_pad], F32, tag=f"str{b}", name=f"str{b}")
                  for b in range(NB)]
        accs = [consts.tile([1, nsub], F32, tag=f"acc{b}", name=f"acc{b}")
                for b in range(NB)]

        def emit_q():
            # q directly in bias layout: per e-tile, stationary = Wq chunk
            # [128, 128], moving = target columns [128, NB] -> [128, NB] psum
            for et in range(ET):
                q_ps = sm_pool.tile([P, NB], F32, tag="small", name="q_ps")
                for dc in range(DC):
                    nc.tensor.matmul(
                        q_ps[:],
                        wqb[:, dc * D + et * P: dc * D + (et + 1) * P],
                        tgt16[:, dc * NB:(dc + 1) * NB],
                        start=(dc == 0),
                        stop=(dc == DC - 1),
                    )
                # bias layout: q_sb[:, b*ET + et]
                for b in range(NB):
                    nc.vector.tensor_copy(q_sb[:, b * ET + et: b * ET + et + 1],
                                          q_ps[:, b:b + 1])

        first = True
        for b in range(NB):
            off = 0
            for sp, w in enumerate(widths):
                vd_ps = vd_pool.tile([P, SW], F32, tag="vd", name="vd_ps")
                ths = {}
                for half in range(2):
                    ets = range(half * 4, half * 4 + 4)
                    k_ps = {et: kps_pool.tile([P, SW], F32, tag="k", name="k_ps")
                            for et in ets}
                    for dc in range(DC):
                        for et in ets:
                            nc.tensor.matmul(
                                k_ps[et][:, :w],
                                wkb[:, dc * D + et * P: dc * D + (et + 1) * P],
                                membs[b][:, dc, off:off + w],
                                start=(dc == 0),
                                stop=(dc == DC - 1),
                            )
                    if first:
                        emit_q()
                        first = False
                    for et in ets:
                        th = th_pool.tile([P, SW], BF16, tag="th", name="th")
                        nc.scalar.activation(
                            th[:, :w], k_ps[et][:, :w], AF.Tanh,
                            bias=q_sb[:, b * ET + et: b * ET + et + 1],
                        )
                        ths[et] = th
                        # v-dot trails the tanh stream on the PE (full-width
                        # stationary: every output row equals the v-dot row)
                        if et >= 2:
                            lag = et - 2
                            nc.tensor.matmul(
                                vd_ps[:, :w], V_all[:, lag * P:(lag + 1) * P],
                                ths[lag][:, :w], start=(lag == 0), stop=False,
                            )
                for et in (ET - 2, ET - 1):
                    nc.tensor.matmul(
                        vd_ps[:, :w], V_all[:, et * P:(et + 1) * P], ths[et][:, :w],
                        start=False, stop=False,
                    )
                nc.tensor.matmul(
                    vd_ps[:, :w], one1p[:],
                    mneg_sb[:, b * s_pad + off: b * s_pad + off + w],
                    start=False, stop=True,
                )
                nc.scalar.activation(
                    scores[b][:, off:off + w], vd_ps[0:1, :w], AF.Exp,
                    accum_out=accs[b][:, sp:sp + 1],
                )
                off += w

        # --- softmax normalization per batch (compact row) ---
        for b in range(NB):
            tot = fin_pool.tile([1, 1], F32, tag="tot", name="tot")
            nc.vector.reduce_sum(tot[:], accs[b][:], axis=mybir.AxisListType.X)
            recip = fin_pool.tile([1, 1], F32, tag="recip", name="recip")
            nc.vector.reciprocal(recip[:], tot[:])
            outs = fin_pool.tile([1, s_pad], F32, tag="outs", name="outs")
            nc.scalar.activation(outs[:], scores[b][:], AF.Copy, scale=recip[:, 0:1])
            nc.sync.dma_start(out[b:b + 1, :], outs[:])

    nc.compile()
    return nc


def get_program(s_pad=None):
    assert s_pad is not None
    if s_pad not in _CACHE:
        _CACHE[s_pad] = _build_program(s_pad)
    return _CACHE[s_pad]


def prepare_in_maps(memory, target, memory_mask, Wq, Wk, v):
    memory = np.asarray(memory, dtype=np.float32)
    target = np.asarray(target, dtype=np.float32)
    Wq = np.asarray(Wq, dtype=np.float32)
    Wk = np.asarray(Wk, dtype=np.float32)
    v = np.asarray(v, dtype=np.float32)
    mask = np.asarray(memory_mask)

    keep_bool = ~mask                                                # [B, S]
    max_kept = int(keep_bool.sum(1).max())
    s_pad = max(512, ((max_kept + 127) // 128) * 128)

    memT = memory.transpose(0, 2, 1)                                 # [B, D, S] view
    kept_pad = np.empty((B, s_pad), dtype=np.int64)
    kept_count = np.empty(B, dtype=np.int64)
    for b in range(B):
        k = np.flatnonzero(keep_bool[b])
        kept_count[b] = len(k)
        kept_pad[b, :len(k)] = k
        kept_pad[b, len(k):] = k[0]  # pad data: duplicate first kept column
    # partition-major compact memory: memC[b, p, dc, s] = memory[b, kept[s], dc*128+p]
    memC = np.empty((B, D, s_pad), dtype=BF16NP)
    for b in range(B):
        memC[b] = memT[b][:, kept_pad[b]]
    memC = np.ascontiguousarray(
        memC.reshape(B, DC, P, s_pad).transpose(0, 2, 1, 3))         # [B, P, DC, s]

    # compact pad mask: 0 at kept positions, -1e9 at pads (pads then produce
    # exp(-1e9) == 0 and never pollute the softmax sum)
    mnegC = np.zeros((B, s_pad), dtype=np.float32)
    for b in range(B):
        mnegC[b, kept_count[b]:] = -1e9
    mnegC = mnegC.astype(BF16NP)

    wkT = np.ascontiguousarray(Wk.T).astype(BF16NP)                  # [D, D]
    wqT = np.ascontiguousarray(Wq.T).astype(BF16NP)                  # [D, D]
    tgtT = np.ascontiguousarray(target.T).astype(BF16NP)             # [D, B]
    vTh = np.ascontiguousarray(v.reshape(ET, P).T)                   # [P, ET] fp32

    in_maps = [
        {
            "memC": np.ascontiguousarray(memC[c * NB:(c + 1) * NB]),
            "wkT": wkT,
            "wqT": wqT,
            "tgtT": np.ascontiguousarray(tgtT[:, c * NB:(c + 1) * NB]),
            "vT": vTh,
            "mneg": np.ascontiguousarray(mnegC[c * NB:(c + 1) * NB]),
        }
        for c in range(N_CORES)
    ]
    global _LAST_META
    _LAST_META = (kept_pad, kept_count, s_pad)
    return in_maps, s_pad


_LAST_META = None


def gather_output(results, meta=None):
    kept_pad, kept_count, s_pad = meta if meta is not None else _LAST_META
    out = np.zeros((B, S), dtype=np.float32)
    for c in range(N_CORES):
        vals = results[c]["out"].reshape(NB, s_pad)
        for bb in range(NB):
            b = c * NB + bb
            kc = kept_count[b]
            out[b, kept_pad[b, :kc]] = vals[bb, :kc]
    return out


def kernel(memory, target, memory_mask, Wq, Wk, v):
    from concourse.bass_utils import run_bass_kernel_spmd

    in_maps, s_pad = prepare_in_maps(memory, target, memory_mask, Wq, Wk, v)
    nc = get_program(s_pad=s_pad)
    res = run_bass_kernel_spmd(nc, in_maps, list(range(N_CORES)))
    return gather_output(res.results)
